# revision 17
# baseline (speedup 1.0000x reference)
"""Causal self-attention (B=2, L=2048, E=1024, H=16) on 8 TRN2 NeuronCores.

Sharding: core c = b*4 + g handles batch b and heads 4g..4g+3.
All matmuls run in float32r (TF32-like, ~1e-4 rel err, 4x fp32 speed).

Orientation trick: everything is computed transposed (features on
partitions, sequence on free axis) so that
  - QKV projection bias is a per-partition scalar (free via ACT drain)
  - scores S^T = (K^T)^T-matmul with Q^T needs no transposes
  - softmax denominator comes free from a ones-column appended to V
  - the W_o projection consumes Y^T directly
The only PE transposes are V^T -> V (needed as PV matmul stationary).

Causal masking: upper-triangular j-tiles are skipped entirely; the 4
diagonal-straddling tiles per i-tile get a host-precomputed 0/1 mask
multiplied in after exp. Rows i >= mask_len[b] must attend uniformly
to ALL positions (reference semantics), handled exactly by blending in
mean(V) columns via a K=1 outer-product matmul.
"""

import numpy as np

import concourse.bacc as bacc
import concourse.mybir as mybir
import concourse.tile as tile
from concourse.bass_utils import run_bass_kernel_spmd
from concourse.masks import make_identity

B, L, E, H, D = 2, 2048, 1024, 16, 64
G = 4          # head-groups (tensor-parallel degree)
HG = H // G    # heads per core
NC = 8
NJ = L // 128  # 16 j-tiles
NI = L // 512  # 4 i-tiles
KC = E // 128  # 8 contraction chunks

F32 = mybir.dt.float32
F32R = mybir.dt.float32r

_CACHED_NC = None


def _build():
    nc = bacc.Bacc("TRN2", target_bir_lowering=False, debug=False, num_devices=NC)

    xt = nc.dram_tensor("xt", [E, L], F32R, kind="ExternalInput").ap()
    wqkv = nc.dram_tensor("wqkv", [E, 3 * HG * D], F32R, kind="ExternalInput").ap()
    wo = nc.dram_tensor("wo", [E, HG * D], F32R, kind="ExternalInput").ap()
    ball = nc.dram_tensor("ball", [3 * HG * D], F32, kind="ExternalInput").ap()
    bo = nc.dram_tensor("bo", [HG * D], F32, kind="ExternalInput").ap()
    invsel = nc.dram_tensor("invsel", [L], F32R, kind="ExternalInput").ap()
    dmask = nc.dram_tensor("dmask", [4, 128, 512], F32, kind="ExternalInput").ap()
    outp = nc.dram_tensor("outp", [HG * D, L], F32, kind="ExternalOutput").ap()

    yt_loc = nc.dram_tensor("yt_loc", [HG, D, L], F32R)
    yt_all = nc.dram_tensor("yt_all", [HG, G * D, L], F32R)

    with tile.TileContext(nc) as tc:
        _emit(nc, tc, xt, wqkv, wo, ball, bo, invsel, dmask, outp,
              yt_loc, yt_all)

    nc.compile()
    return nc


def _emit(nc, tc, xt, wqkv, wo, ball, bo, invsel, dmask, outp,
          yt_loc, yt_all):
    from contextlib import ExitStack

    ctx = ExitStack()
    with ctx:
        pW1 = ctx.enter_context(tc.tile_pool(name="pW1", bufs=1))
        pW2 = ctx.enter_context(tc.tile_pool(name="pW2", bufs=1))
        pS = ctx.enter_context(tc.tile_pool(name="pS", bufs=3))
        pRow = ctx.enter_context(tc.tile_pool(name="pRow", bufs=2))
        pPs = ctx.enter_context(tc.tile_pool(name="pPs", bufs=2, space="PSUM"))
        pPy = ctx.enter_context(tc.tile_pool(name="pPy", bufs=2, space="PSUM"))
        pPb = ctx.enter_context(tc.tile_pool(name="pPb", bufs=2, space="PSUM"))

        # ---- long-lived residents ----
        w_qkv_sb = pW1.tile([128, KC, 6 * 128], F32R)       # [p, kc, m]
        nc.sync.dma_start(w_qkv_sb[:], wqkv.rearrange("(a p) m -> p a m", p=128))
        b_sb = pW1.tile([128, 6], F32)
        nc.sync.dma_start(b_sb[:], ball.rearrange("(a p) -> p a", p=128))
        w_o_sb = pW2.tile([128, KC, 2 * 128], F32R)
        nc.sync.dma_start(w_o_sb[:], wo.rearrange("(a p) m -> p a m", p=128))
        bo_sb = pW2.tile([128, 2], F32)
        nc.sync.dma_start(bo_sb[:], bo.rearrange("(a p) -> p a", p=128))
        ytn_sb = pW2.tile([128, 2, L], F32R)  # Y^T normalized

        with tc.tile_pool(name="pBig", bufs=1) as pBig, ExitStack() as ctx2:
            qkvT = pBig.tile([128, 6, L], F32R)   # [q(4x64) | k(4x64) | v(4x64)]

            # ---- Stage A: QKV^T projection ----
            with tc.tile_pool(name="pX", bufs=1) as pX:
                xfull = pX.tile([128, KC, L], F32R)
                for kc in range(KC):
                    nc.sync.dma_start(
                        xfull[:, kc, :], xt[kc * 128:(kc + 1) * 128, :])
                for mt in [0, 2, 4, 1, 3, 5]:
                    for ntp in range(NI // 2):
                        ps = pPs.tile([128, 1024], F32, tag="psS")
                        for half in range(2):
                            nt = 2 * ntp + half
                            for kc in range(KC):
                                nc.tensor.matmul(
                                    ps[:, half * 512:(half + 1) * 512],
                                    w_qkv_sb[:, kc, mt * 128:(mt + 1) * 128],
                                    xfull[:, kc, nt * 512:(nt + 1) * 512],
                                    start=(kc == 0), stop=(kc == KC - 1),
                                )
                        nc.scalar.activation(
                            qkvT[:, mt, 2 * ntp * 512:(2 * ntp + 2) * 512], ps[:],
                            mybir.ActivationFunctionType.Identity,
                            bias=b_sb[:, mt:mt + 1],
                        )

            # ---- post-A residents (reuse the freed x-buffer space) ----
            pMisc = ctx2.enter_context(tc.tile_pool(name="pMisc", bufs=1))
            v_sb = pMisc.tile([128, NJ, HG, 65], F32R)
            dmask_sb = pMisc.tile([128, 4, 512], F32)
            nc.sync.dma_start(dmask_sb[:], dmask.rearrange("a p m -> p a m"))
            invsel_sb = pMisc.tile([1, L], F32R)
            nc.sync.dma_start(invsel_sb[:], invsel.rearrange("(a m) -> a m", a=1))

            # constants: build in f32, round into f32r via DVE copies
            ident_f32 = pMisc.tile([128, 64], F32)
            nc.gpsimd.memset(ident_f32[:], 0.0)
            make_identity(nc, ident_f32[0:64, :], nomemset=True)
            make_identity(nc, ident_f32[64:128, :], nomemset=True)
            ident = pMisc.tile([128, 64], F32R)  # two stacked 64x64 identities
            nc.vector.tensor_copy(ident[:], ident_f32[:])
            ones_f32 = pMisc.tile([128, 64], F32)
            nc.vector.memset(ones_f32[:], 1.0)
            ones_col = pMisc.tile([128, 1], F32R)
            nc.vector.tensor_copy(ones_col[:], ones_f32[:, 0:1])
            ones_row = pMisc.tile([1, 64], F32R)
            nc.vector.tensor_copy(ones_row[:], ones_f32[0:1, :])
            big_f32 = pMisc.tile([1, 64], F32)
            nc.vector.memset(big_f32[:], 1e30)
            big_row = pMisc.tile([1, 64], F32R)
            nc.vector.tensor_copy(big_row[:], big_f32[:])
            for jt in range(NJ):  # colsum ones column of V
                nc.vector.tensor_copy(v_sb[:, jt, :, 64], ones_f32[:, 0:HG])
            meanv_sb = pMisc.tile([1, HG, 64], F32R)

            # ---- Stage B: V^T -> V transposes (heads interleaved so
            # head 0/1 V is ready while stage A finishes heads 2/3) ----
            for h in [0, 1, 2, 3]:
                mt = 4 + h // 2
                off = 64 * (h % 2)
                for jt in range(NJ):
                    pt = pPy.tile([128, 64], F32R, tag="psY")
                    nc.tensor.matmul(
                        pt[:],
                        qkvT[off:off + 64, mt, jt * 128:(jt + 1) * 128],
                        ident[off:off + 64, :],
                        start=True, stop=True, is_transpose=True,
                    )
                    nc.vector.tensor_copy(v_sb[:, jt, h, 0:64], pt[:])

            # ---- Stage C: attention ----
            # meanV for all heads: one ones-matmul per j-tile
            pm = pPb.tile([1, HG * 64], F32, tag="psB")
            for jt in range(NJ):
                nc.tensor.matmul(
                    pm[:], ones_col[:],
                    v_sb[:, jt, :, 0:64],
                    start=(jt == 0), stop=(jt == NJ - 1),
                )
            nc.scalar.activation(
                meanv_sb[0:1, :, :], pm[:],
                mybir.ActivationFunctionType.Copy, scale=1.0 / L,
            )

            for h in range(HG):
                q_mt, q_off = h // 2, 64 * (h % 2)
                k_mt, k_off = 2 + h // 2, 64 * (h % 2)

                for it in range(NI):
                    n_j = 4 * it + 4
                    py = pPy.tile([65, 512], F32, tag="psY")
                    for jp in range(n_j // 2):
                        ps = pPs.tile([128, 1024], F32, tag="psS")
                        for half in range(2):
                            jt = 2 * jp + half
                            nc.tensor.matmul(
                                ps[:, half * 512:(half + 1) * 512],
                                qkvT[k_off:k_off + 64, k_mt,
                                     jt * 128:(jt + 1) * 128],
                                qkvT[q_off:q_off + 64, q_mt,
                                     it * 512:(it + 1) * 512],
                                start=True, stop=True,
                            )
                        es = pS.tile([128, 1024], F32R, tag="expS")
                        nc.scalar.activation(
                            es[:], ps[:],
                            mybir.ActivationFunctionType.Exp, scale=0.125,
                        )
                        for half in range(2):
                            jt = 2 * jp + half
                            p = jt - 4 * it
                            esh = es[:, half * 512:(half + 1) * 512]
                            if p >= 0:  # diagonal-straddling tile
                                nc.gpsimd.tensor_mul(
                                    esh, esh.bitcast(F32), dmask_sb[:, p, :])
                            nc.tensor.matmul(
                                py[:], v_sb[:, jt, h, :], esh,
                                start=(jt == 0), stop=(jt == n_j - 1),
                            )

                    # normalization + invalid-row blend:
                    # denom_bc[d,i] = colsum[i] + 1e30*invsel[i]  (invalid i
                    # -> recip ~1e-30 -> Y term vanishes; meanV term added)
                    cs = pRow.tile([1, 512], F32R, tag="cs")
                    nc.vector.tensor_copy(cs[:], py[64:65, :])
                    pb1 = pPb.tile([64, 512], F32, tag="psB")
                    nc.tensor.matmul(pb1[:], ones_row[:], cs[:],
                                     start=True, stop=False)
                    nc.tensor.matmul(
                        pb1[:], big_row[:],
                        invsel_sb[:, it * 512:(it + 1) * 512],
                        start=False, stop=True,
                    )
                    rbc = pS.tile([64, 512], F32, tag="rbc")
                    nc.vector.reciprocal_approx_fast(rbc[:], pb1[:])
                    pb2 = pPb.tile([64, 512], F32, tag="psB")
                    nc.tensor.matmul(
                        pb2[:], meanv_sb[:, h, :],
                        invsel_sb[:, it * 512:(it + 1) * 512],
                        start=True, stop=True,
                    )
                    dst = ytn_sb[64 * (h % 2):64 * (h % 2) + 64, h // 2,
                                 it * 512:(it + 1) * 512]
                    nc.vector.tensor_mul(dst, py[0:64, :], rbc[:])
                    nc.vector.tensor_add(dst, dst.bitcast(F32), pb2[:])

                # per-head AllGather, overlapped with later heads' compute
                nc.sync.dma_start(
                    yt_loc[h],
                    ytn_sb[64 * (h % 2):64 * (h % 2) + 64, h // 2, :],
                )
                nc.gpsimd.collective_compute(
                    "AllGather",
                    mybir.AluOpType.bypass,
                    replica_groups=[[0, 1, 2, 3], [4, 5, 6, 7]],
                    ins=[yt_loc[h]],
                    outs=[yt_all[h]],
                )

        # ---- Stage D: output projection (W_o rows host-permuted to
        # match the (h_local, g, d) gathered row order) ----
        with tc.tile_pool(name="pYt", bufs=1) as pYt:
            yt_sb = pYt.tile([128, KC, L], F32R)
            for h in range(HG):
                nc.sync.dma_start(
                    yt_sb[:, 2 * h:2 * h + 2, :],
                    yt_all[h].rearrange("(a p) m -> p a m", p=128),
                )
            for ot in range(2):
                for ntp in range(NI // 2):
                    po = pPs.tile([128, 1024], F32, tag="psS")
                    for half in range(2):
                        nt = 2 * ntp + half
                        for kc in range(KC):
                            nc.tensor.matmul(
                                po[:, half * 512:(half + 1) * 512],
                                w_o_sb[:, kc, ot * 128:(ot + 1) * 128],
                                yt_sb[:, kc, nt * 512:(nt + 1) * 512],
                                start=(kc == 0), stop=(kc == KC - 1),
                            )
                    ob = pS.tile([128, 1024], F32, tag="outsb")
                    nc.scalar.activation(
                        ob[:], po[:], mybir.ActivationFunctionType.Identity,
                        bias=bo_sb[:, ot:ot + 1],
                    )
                    nc.sync.dma_start(
                        outp[ot * 128:(ot + 1) * 128,
                             2 * ntp * 512:(2 * ntp + 2) * 512], ob[:])


def _prep_inputs(x, W_qkv, b_qkv, W_o, b_o, mask_len):
    x = np.asarray(x, dtype=np.float32)
    W_qkv = np.asarray(W_qkv, dtype=np.float32)
    b_qkv = np.asarray(b_qkv, dtype=np.float32)
    W_o = np.asarray(W_o, dtype=np.float32)
    b_o = np.asarray(b_o, dtype=np.float32)
    mask_len = np.asarray(mask_len)

    # per-head q/k/v column blocks of W_qkv (packed per-head [q|k|v] of 3D)
    wq = [W_qkv[:, 3 * D * h:3 * D * h + D] for h in range(H)]
    wk = [W_qkv[:, 3 * D * h + D:3 * D * h + 2 * D] for h in range(H)]
    wv = [W_qkv[:, 3 * D * h + 2 * D:3 * D * h + 3 * D] for h in range(H)]
    bq = [b_qkv[3 * D * h:3 * D * h + D] for h in range(H)]
    bk = [b_qkv[3 * D * h + D:3 * D * h + 2 * D] for h in range(H)]
    bv = [b_qkv[3 * D * h + 2 * D:3 * D * h + 3 * D] for h in range(H)]

    dmask = np.zeros((4, 128, 512), dtype=np.float32)
    jj = np.arange(128)[:, None]
    ii = np.arange(512)[None, :]
    for p in range(4):
        dmask[p] = (ii >= 128 * p + jj).astype(np.float32)

    in_maps = []
    for c in range(NC):
        b, g = divmod(c, G)
        hs = list(range(HG * g, HG * g + HG))
        wqkv_c = np.ascontiguousarray(np.concatenate(
            [wq[h] for h in hs] + [wk[h] for h in hs] + [wv[h] for h in hs],
            axis=1))
        ball_c = np.ascontiguousarray(np.concatenate(
            [bq[h] for h in hs] + [bk[h] for h in hs] + [bv[h] for h in hs]))
        # permute W_o rows to the per-head-gathered order (h_local, g, d)
        perm = np.concatenate(
            [np.arange(64 * (4 * gg + hh), 64 * (4 * gg + hh) + 64)
             for hh in range(HG) for gg in range(G)])
        wo_c = np.ascontiguousarray(W_o[perm][:, 256 * g:256 * (g + 1)])
        bo_c = np.ascontiguousarray(b_o[256 * g:256 * (g + 1)])
        xt_b = np.ascontiguousarray(x[b].T)
        sel_b = (np.arange(L) < int(mask_len[b])).astype(np.float32)
        in_maps.append({
            "xt": xt_b,
            "wqkv": wqkv_c,
            "wo": wo_c,
            "ball": ball_c,
            "bo": bo_c,
            "invsel": np.ascontiguousarray(1.0 - sel_b),
            "dmask": dmask,
        })
    return in_maps


def kernel(x, W_qkv, b_qkv, W_o, b_o, mask_len):
    global _CACHED_NC
    in_maps = _prep_inputs(x, W_qkv, b_qkv, W_o, b_o, mask_len)
    if _CACHED_NC is None:
        _CACHED_NC = _build()
    res = run_bass_kernel_spmd(_CACHED_NC, in_maps, core_ids=list(range(NC)))
    out = np.empty((B, L, E), dtype=np.float32)
    for b in range(B):
        ot = np.concatenate(
            [res.results[G * b + g]["outp"] for g in range(G)], axis=0)
        out[b] = ot.T
    return out


# revision 18
# speedup vs baseline: 1.2417x; 1.2417x over previous
"""Causal self-attention (B=2, L=2048, E=1024, H=16) on 8 TRN2 NeuronCores.

Sharding: core c = b*4 + g handles batch b and heads 4g..4g+3.
All matmuls run in float32r (TF32-like, ~1e-4 rel err, 4x fp32 speed).

Orientation trick: everything is computed transposed (features on
partitions, sequence on free axis) so that
  - QKV projection bias is a per-partition scalar (free via ACT drain)
  - scores S^T = (K^T)^T-matmul with Q^T needs no transposes
  - softmax denominator comes free from a ones-column appended to V
  - the W_o projection consumes Y^T directly
The only PE transposes are V^T -> V (needed as PV matmul stationary).

Causal masking: upper-triangular j-tiles are skipped entirely; the 4
diagonal-straddling tiles per i-tile get a host-precomputed 0/1 mask
multiplied in after exp. Rows i >= mask_len[b] must attend uniformly
to ALL positions (reference semantics), handled exactly by blending in
mean(V) columns via a K=1 outer-product matmul.
"""

import os

import numpy as np

import concourse.bacc as bacc
import concourse.mybir as mybir
import concourse.tile as tile
from concourse.bass_utils import run_bass_kernel_spmd
from concourse.masks import make_identity

B, L, E, H, D = 2, 2048, 1024, 16, 64
G = 4          # head-groups (tensor-parallel degree)
HG = H // G    # heads per core
NC = 8
NJ = L // 128  # 16 j-tiles
NI = L // 512  # 4 i-tiles
KC = E // 128  # 8 contraction chunks

F32 = mybir.dt.float32
F32R = mybir.dt.float32r
BF16 = mybir.dt.bfloat16

# matmul compute dtype: "f32r" (TF32-like, ~2e-4 rel err) or "bf16"
MM_MODE = os.environ.get("KERNEL_DTYPE", "f32r")
MDT = BF16 if MM_MODE == "bf16" else F32R
MASK_DT = BF16 if MM_MODE == "bf16" else F32

_CACHED_NC = None


def _build():
    nc = bacc.Bacc("TRN2", target_bir_lowering=False, debug=False, num_devices=NC)

    xt = nc.dram_tensor("xt", [E, L], MDT, kind="ExternalInput").ap()
    wqkv = nc.dram_tensor("wqkv", [E, 3 * HG * D], MDT, kind="ExternalInput").ap()
    wo = nc.dram_tensor("wo", [E, HG * D], MDT, kind="ExternalInput").ap()
    ball = nc.dram_tensor("ball", [3 * HG * D], F32, kind="ExternalInput").ap()
    bo = nc.dram_tensor("bo", [HG * D], F32, kind="ExternalInput").ap()
    invsel = nc.dram_tensor("invsel", [L], MDT, kind="ExternalInput").ap()
    dmask = nc.dram_tensor("dmask", [4, 128, 512], MASK_DT, kind="ExternalInput").ap()
    outp = nc.dram_tensor("outp", [HG * D, L], F32, kind="ExternalOutput").ap()

    yt_loc = nc.dram_tensor("yt_loc", [HG, D, L], MDT)
    yt_all = nc.dram_tensor("yt_all", [HG, G * D, L], MDT)

    with tile.TileContext(nc) as tc:
        _emit(nc, tc, xt, wqkv, wo, ball, bo, invsel, dmask, outp,
              yt_loc, yt_all)

    nc.compile()
    return nc


def _emit(nc, tc, xt, wqkv, wo, ball, bo, invsel, dmask, outp,
          yt_loc, yt_all):
    from contextlib import ExitStack

    ctx = ExitStack()
    with ctx:
        pW1 = ctx.enter_context(tc.tile_pool(name="pW1", bufs=1))
        pW2 = ctx.enter_context(tc.tile_pool(name="pW2", bufs=1))
        pS = ctx.enter_context(tc.tile_pool(name="pS", bufs=3))
        pRow = ctx.enter_context(tc.tile_pool(name="pRow", bufs=2))
        pPs = ctx.enter_context(tc.tile_pool(name="pPs", bufs=2, space="PSUM"))
        pPy = ctx.enter_context(tc.tile_pool(name="pPy", bufs=2, space="PSUM"))
        pPb = ctx.enter_context(tc.tile_pool(name="pPb", bufs=2, space="PSUM"))

        # ---- long-lived residents ----
        w_qkv_sb = pW1.tile([128, KC, 6 * 128], MDT)       # [p, kc, m]
        nc.sync.dma_start(w_qkv_sb[:], wqkv.rearrange("(a p) m -> p a m", p=128))
        b_sb = pW1.tile([128, 6], F32)
        nc.sync.dma_start(b_sb[:], ball.rearrange("(a p) -> p a", p=128))
        w_o_sb = pW2.tile([128, KC, 2 * 128], MDT)
        nc.sync.dma_start(w_o_sb[:], wo.rearrange("(a p) m -> p a m", p=128))
        bo_sb = pW2.tile([128, 2], F32)
        nc.sync.dma_start(bo_sb[:], bo.rearrange("(a p) -> p a", p=128))
        ytn_sb = pW2.tile([128, 2, L], MDT)  # Y^T normalized

        with tc.tile_pool(name="pBig", bufs=1) as pBig, ExitStack() as ctx2:
            qkvT = pBig.tile([128, 6, L], MDT)   # [q(4x64) | k(4x64) | v(4x64)]

            # ---- Stage A: QKV^T projection ----
            with tc.tile_pool(name="pX", bufs=1) as pX:
                xfull = pX.tile([128, KC, L], MDT)
                for kc in range(KC):
                    nc.sync.dma_start(
                        xfull[:, kc, :], xt[kc * 128:(kc + 1) * 128, :])
                for mt in [0, 2, 4, 1, 3, 5]:
                    for ntp in range(NI // 2):
                        ps = pPs.tile([128, 1024], F32, tag="psS")
                        for half in range(2):
                            nt = 2 * ntp + half
                            for kc in range(KC):
                                nc.tensor.matmul(
                                    ps[:, half * 512:(half + 1) * 512],
                                    w_qkv_sb[:, kc, mt * 128:(mt + 1) * 128],
                                    xfull[:, kc, nt * 512:(nt + 1) * 512],
                                    start=(kc == 0), stop=(kc == KC - 1),
                                )
                        nc.scalar.activation(
                            qkvT[:, mt, 2 * ntp * 512:(2 * ntp + 2) * 512], ps[:],
                            mybir.ActivationFunctionType.Identity,
                            bias=b_sb[:, mt:mt + 1],
                        )

            # ---- post-A residents (reuse the freed x-buffer space) ----
            pMisc = ctx2.enter_context(tc.tile_pool(name="pMisc", bufs=1))
            v_sb = pMisc.tile([128, NJ, HG, 65], MDT)
            dmask_sb = pMisc.tile([128, 4, 512], MASK_DT)
            nc.sync.dma_start(dmask_sb[:], dmask.rearrange("a p m -> p a m"))
            invsel_sb = pMisc.tile([1, L], MDT)
            nc.sync.dma_start(invsel_sb[:], invsel.rearrange("(a m) -> a m", a=1))

            # constants: build in f32, round into f32r via DVE copies
            ident_f32 = pMisc.tile([128, 64], F32)
            nc.gpsimd.memset(ident_f32[:], 0.0)
            make_identity(nc, ident_f32[0:64, :], nomemset=True)
            make_identity(nc, ident_f32[64:128, :], nomemset=True)
            ident = pMisc.tile([128, 64], MDT)  # two stacked 64x64 identities
            nc.vector.tensor_copy(ident[:], ident_f32[:])
            ones_f32 = pMisc.tile([128, 64], F32)
            nc.vector.memset(ones_f32[:], 1.0)
            ones_col = pMisc.tile([128, 1], MDT)
            nc.vector.tensor_copy(ones_col[:], ones_f32[:, 0:1])
            ones_row = pMisc.tile([1, 64], MDT)
            nc.vector.tensor_copy(ones_row[:], ones_f32[0:1, :])
            big_f32 = pMisc.tile([1, 64], F32)
            nc.vector.memset(big_f32[:], 1e30)
            big_row = pMisc.tile([1, 64], MDT)
            nc.vector.tensor_copy(big_row[:], big_f32[:])
            for jt in range(NJ):  # colsum ones column of V
                nc.vector.tensor_copy(v_sb[:, jt, :, 64], ones_f32[:, 0:HG])
            meanv_sb = pMisc.tile([1, HG, 64], MDT)

            # ---- Stage B: V^T -> V transposes (heads interleaved so
            # head 0/1 V is ready while stage A finishes heads 2/3) ----
            for h in [0, 1, 2, 3]:
                mt = 4 + h // 2
                off = 64 * (h % 2)
                for jt in range(NJ):
                    pt = pPy.tile([128, 64], MDT, tag="psY")
                    nc.tensor.matmul(
                        pt[:],
                        qkvT[off:off + 64, mt, jt * 128:(jt + 1) * 128],
                        ident[off:off + 64, :],
                        start=True, stop=True, is_transpose=True,
                    )
                    nc.vector.tensor_copy(v_sb[:, jt, h, 0:64], pt[:])

            # ---- Stage C: attention ----
            # meanV for all heads: one ones-matmul per j-tile
            pm = pPb.tile([1, HG * 64], F32, tag="psB")
            for jt in range(NJ):
                nc.tensor.matmul(
                    pm[:], ones_col[:],
                    v_sb[:, jt, :, 0:64],
                    start=(jt == 0), stop=(jt == NJ - 1),
                )
            nc.scalar.activation(
                meanv_sb[0:1, :, :], pm[:],
                mybir.ActivationFunctionType.Copy, scale=1.0 / L,
            )

            for h in range(HG):
                q_mt, q_off = h // 2, 64 * (h % 2)
                k_mt, k_off = 2 + h // 2, 64 * (h % 2)

                for it in range(NI):
                    n_j = 4 * it + 4
                    py = pPy.tile([65, 512], F32, tag="psY")
                    for jp in range(n_j // 2):
                        ps = pPs.tile([128, 1024], F32, tag="psS")
                        for half in range(2):
                            jt = 2 * jp + half
                            nc.tensor.matmul(
                                ps[:, half * 512:(half + 1) * 512],
                                qkvT[k_off:k_off + 64, k_mt,
                                     jt * 128:(jt + 1) * 128],
                                qkvT[q_off:q_off + 64, q_mt,
                                     it * 512:(it + 1) * 512],
                                start=True, stop=True,
                            )
                        es = pS.tile([128, 1024], MDT, tag="expS")
                        nc.scalar.activation(
                            es[:], ps[:],
                            mybir.ActivationFunctionType.Exp, scale=0.125,
                        )
                        for half in range(2):
                            jt = 2 * jp + half
                            p = jt - 4 * it
                            esh = es[:, half * 512:(half + 1) * 512]
                            if p >= 0:  # diagonal-straddling tile
                                op_dt = BF16 if MM_MODE == "bf16" else F32
                                nc.gpsimd.tensor_mul(
                                    esh, esh.bitcast(op_dt), dmask_sb[:, p, :])
                            nc.tensor.matmul(
                                py[:], v_sb[:, jt, h, :], esh,
                                start=(jt == 0), stop=(jt == n_j - 1),
                            )

                    # normalization + invalid-row blend:
                    # denom_bc[d,i] = colsum[i] + 1e30*invsel[i]  (invalid i
                    # -> recip ~1e-30 -> Y term vanishes; meanV term added)
                    cs = pRow.tile([1, 512], MDT, tag="cs")
                    nc.vector.tensor_copy(cs[:], py[64:65, :])
                    pb1 = pPb.tile([64, 512], F32, tag="psB")
                    nc.tensor.matmul(pb1[:], ones_row[:], cs[:],
                                     start=True, stop=False)
                    nc.tensor.matmul(
                        pb1[:], big_row[:],
                        invsel_sb[:, it * 512:(it + 1) * 512],
                        start=False, stop=True,
                    )
                    rbc = pS.tile([64, 512], F32, tag="rbc")
                    nc.vector.reciprocal_approx_fast(rbc[:], pb1[:])
                    pb2 = pPb.tile([64, 512], F32, tag="psB")
                    nc.tensor.matmul(
                        pb2[:], meanv_sb[:, h, :],
                        invsel_sb[:, it * 512:(it + 1) * 512],
                        start=True, stop=True,
                    )
                    dst = ytn_sb[64 * (h % 2):64 * (h % 2) + 64, h // 2,
                                 it * 512:(it + 1) * 512]
                    nc.vector.tensor_mul(dst, py[0:64, :], rbc[:])
                    nc.vector.tensor_add(
                        dst, dst.bitcast(BF16 if MM_MODE == "bf16" else F32),
                        pb2[:])

                # per-head AllGather, overlapped with later heads' compute
                nc.sync.dma_start(
                    yt_loc[h],
                    ytn_sb[64 * (h % 2):64 * (h % 2) + 64, h // 2, :],
                )
                nc.gpsimd.collective_compute(
                    "AllGather",
                    mybir.AluOpType.bypass,
                    replica_groups=[[0, 1, 2, 3], [4, 5, 6, 7]],
                    ins=[yt_loc[h]],
                    outs=[yt_all[h]],
                )

        # ---- Stage D: output projection (W_o rows host-permuted to
        # match the (h_local, g, d) gathered row order) ----
        with tc.tile_pool(name="pYt", bufs=1) as pYt:
            yt_sb = pYt.tile([128, KC, L], MDT)
            for h in range(HG):
                nc.sync.dma_start(
                    yt_sb[:, 2 * h:2 * h + 2, :],
                    yt_all[h].rearrange("(a p) m -> p a m", p=128),
                )
            for ot in range(2):
                for ntp in range(NI // 2):
                    po = pPs.tile([128, 1024], F32, tag="psS")
                    for half in range(2):
                        nt = 2 * ntp + half
                        for kc in range(KC):
                            nc.tensor.matmul(
                                po[:, half * 512:(half + 1) * 512],
                                w_o_sb[:, kc, ot * 128:(ot + 1) * 128],
                                yt_sb[:, kc, nt * 512:(nt + 1) * 512],
                                start=(kc == 0), stop=(kc == KC - 1),
                            )
                    ob = pS.tile([128, 1024], F32, tag="outsb")
                    nc.scalar.activation(
                        ob[:], po[:], mybir.ActivationFunctionType.Identity,
                        bias=bo_sb[:, ot:ot + 1],
                    )
                    nc.sync.dma_start(
                        outp[ot * 128:(ot + 1) * 128,
                             2 * ntp * 512:(2 * ntp + 2) * 512], ob[:])


def _prep_inputs(x, W_qkv, b_qkv, W_o, b_o, mask_len):
    x = np.asarray(x, dtype=np.float32)
    W_qkv = np.asarray(W_qkv, dtype=np.float32)
    b_qkv = np.asarray(b_qkv, dtype=np.float32)
    W_o = np.asarray(W_o, dtype=np.float32)
    b_o = np.asarray(b_o, dtype=np.float32)
    mask_len = np.asarray(mask_len)

    # per-head q/k/v column blocks of W_qkv (packed per-head [q|k|v] of 3D)
    wq = [W_qkv[:, 3 * D * h:3 * D * h + D] for h in range(H)]
    wk = [W_qkv[:, 3 * D * h + D:3 * D * h + 2 * D] for h in range(H)]
    wv = [W_qkv[:, 3 * D * h + 2 * D:3 * D * h + 3 * D] for h in range(H)]
    bq = [b_qkv[3 * D * h:3 * D * h + D] for h in range(H)]
    bk = [b_qkv[3 * D * h + D:3 * D * h + 2 * D] for h in range(H)]
    bv = [b_qkv[3 * D * h + 2 * D:3 * D * h + 3 * D] for h in range(H)]

    dmask = np.zeros((4, 128, 512), dtype=np.float32)
    jj = np.arange(128)[:, None]
    ii = np.arange(512)[None, :]
    for p in range(4):
        dmask[p] = (ii >= 128 * p + jj).astype(np.float32)

    import ml_dtypes
    mm_np = ml_dtypes.bfloat16 if MM_MODE == "bf16" else np.float32

    in_maps = []
    for c in range(NC):
        b, g = divmod(c, G)
        hs = list(range(HG * g, HG * g + HG))
        wqkv_c = np.ascontiguousarray(np.concatenate(
            [wq[h] for h in hs] + [wk[h] for h in hs] + [wv[h] for h in hs],
            axis=1))
        ball_c = np.ascontiguousarray(np.concatenate(
            [bq[h] for h in hs] + [bk[h] for h in hs] + [bv[h] for h in hs]))
        # permute W_o rows to the per-head-gathered order (h_local, g, d)
        perm = np.concatenate(
            [np.arange(64 * (4 * gg + hh), 64 * (4 * gg + hh) + 64)
             for hh in range(HG) for gg in range(G)])
        wo_c = np.ascontiguousarray(W_o[perm][:, 256 * g:256 * (g + 1)])
        bo_c = np.ascontiguousarray(b_o[256 * g:256 * (g + 1)])
        xt_b = np.ascontiguousarray(x[b].T)
        sel_b = (np.arange(L) < int(mask_len[b])).astype(np.float32)
        in_maps.append({
            "xt": xt_b.astype(mm_np),
            "wqkv": wqkv_c.astype(mm_np),
            "wo": wo_c.astype(mm_np),
            "ball": ball_c,
            "bo": bo_c,
            "invsel": np.ascontiguousarray(1.0 - sel_b).astype(mm_np),
            "dmask": dmask.astype(mm_np),
        })
    return in_maps


def kernel(x, W_qkv, b_qkv, W_o, b_o, mask_len):
    global _CACHED_NC
    in_maps = _prep_inputs(x, W_qkv, b_qkv, W_o, b_o, mask_len)
    if _CACHED_NC is None:
        _CACHED_NC = _build()
    res = run_bass_kernel_spmd(_CACHED_NC, in_maps, core_ids=list(range(NC)))
    out = np.empty((B, L, E), dtype=np.float32)
    for b in range(B):
        ot = np.concatenate(
            [res.results[G * b + g]["outp"] for g in range(G)], axis=0)
        out[b] = ot.T
    return out
